# revision 23
# baseline (speedup 1.0000x reference)
"""DeepseekV3 MoE kernel for 8 Trainium2 NeuronCores (expert-parallel, fp8).

Strategy:
  - Host: grouped top-k gating (exact replica of the reference jax ops, on CPU),
    token dispatch (gather tokens per expert, zero-padded to CAP=128 slots;
    tokens beyond 128 per expert fall back to exact host numpy — ~260 tokens).
  - Device (SPMD over 8 cores): core c owns 8 routed experts (e = 8c..8c+7)
    plus a (token-half, intermediate-quarter) tile of the two shared experts.
    The gate_up matmuls run double-pumped (DoubleRow, e4m3 weights x e4m3
    tokens, 2 k-tiles per instruction); the down-proj runs normal-rate with
    e3m4 weights against bf16 hidden; y ships as e3m4. Shared experts stay
    bf16 (their error is not attenuated by the combine weights). All scales
    are powers of two, folded into the silu input scale and the host-side
    combine weights. Matmuls are token-stationary (stationary = [contraction,
    128 tokens], moving = weights 512 cols/instruction); the hidden transpose
    for the down-proj runs on the PE against a DMA'd identity. A memset-fed
    warm-up matmul stream ramps the PE p-state before the first DMAs land,
    and shared-expert compute fills the slots where the weight stream lags.
  - Host: descale y rows, multiply by combine weight, place at flat (t,k)
    positions and sum over k; add host-computed overflow rows; sum shared.

Shapes (hardcoded): T=1024, H=1024, I=512, E=64, S=2, G=8, TOPK_GROUP=4, K=8.
"""
import numpy as np
import ml_dtypes
from contextlib import ExitStack

import concourse.bass as bass
from concourse import mybir, tile, bacc
from concourse.bass_utils import run_bass_kernel_spmd

f32 = mybir.dt.float32
bf16 = mybir.dt.bfloat16
fp8 = mybir.dt.float8e3
fp8e4 = mybir.dt.float8e4
AF = mybir.ActivationFunctionType
PM = mybir.MatmulPerfMode
BF = ml_dtypes.bfloat16
E3 = ml_dtypes.float8_e3m4
E4 = ml_dtypes.float8_e4m3

T, H, I, E, S = 1024, 1024, 512, 64, 2
G, TOPK_GROUP, K = 8, 4, 8
I2 = 2 * I
N_CORES = 8
E_LOC = E // N_CORES          # 8 experts per core
HT = H // 128                 # 8 k-tiles over hidden dim
IT = I // 128                 # 4 k-tiles over intermediate dim
CAP = 128                     # token slots per expert (stationary width)
TSH = 512                     # shared: tokens per core (T / 2 token groups)
ISHC = 128                    # shared: intermediate cols per core (I / 4)

E3_MAX = 15.5                 # e3m4 max normal
E4_MAX = 240.0                # TRN e4m3 max normal
SCL_X = 16.0                  # token scale into e4m3
SCL_W = 1024.0                # gate_up weight scale into e4m3
SCL_WD = 128.0                # down weight scale into e3m4
SCL_Y = 16.0                  # y output scale into e3m4
GU_DESCALE = 1.0 / (SCL_X * SCL_W)           # gu psum -> true gate units
Y_DESCALE = SCL_Y / (SCL_X * SCL_W * SCL_WD)  # down psum -> SCL_Y * y

_TRACE = False
_CACHED_NC = None
LAST_RESULTS = None


def _build_nc():
    nc = bacc.Bacc("TRN2", target_bir_lowering=False, debug=False)

    # partition-major layouts: per-partition DRAM runs are 1-8 KB contiguous
    xg_d = nc.dram_tensor("xg", [128, E_LOC, HT, CAP], fp8e4,
                          kind="ExternalInput")
    wgu_d = nc.dram_tensor("wgu", [E_LOC, 128, HT, I2], fp8e4,
                           kind="ExternalInput")
    # down weights packed per expert PAIR for single-trigger loads
    wdp_d = nc.dram_tensor("wdp", [E_LOC // 2, 128, 2, IT, H], fp8,
                           kind="ExternalInput")
    xt_d = nc.dram_tensor("xt", [128, HT, TSH], bf16, kind="ExternalInput")
    swh_d = nc.dram_tensor("swh", [128, S, HT, 2 * ISHC], bf16,
                           kind="ExternalInput")
    sdc_d = nc.dram_tensor("sdc", [128, S, H], bf16, kind="ExternalInput")
    id_d = nc.dram_tensor("ident", [128, 128], bf16, kind="ExternalInput")
    y_d = nc.dram_tensor("y", [E_LOC, 128, H], fp8, kind="ExternalOutput")
    sh_d = nc.dram_tensor("sh", [TSH, H], bf16, kind="ExternalOutput")

    with tile.TileContext(nc) as tc, ExitStack() as ctx:
        const_p = ctx.enter_context(tc.tile_pool(name="const", bufs=1))
        wgu_p = ctx.enter_context(tc.tile_pool(name="wgu", bufs=8))
        wd_p = ctx.enter_context(tc.tile_pool(name="wd", bufs=4))
        act_p = ctx.enter_context(tc.tile_pool(name="act", bufs=3))
        y_p = ctx.enter_context(tc.tile_pool(name="y", bufs=3))
        sh_p = ctx.enter_context(tc.tile_pool(name="sh", bufs=2))
        ps_gu = ctx.enter_context(tc.tile_pool(name="psgu", bufs=4, space="PSUM"))
        ps_y = ctx.enter_context(tc.tile_pool(name="psy", bufs=2, space="PSUM"))
        ps_tr = ctx.enter_context(tc.tile_pool(name="pstr", bufs=2, space="PSUM"))

        # zero tile for PE warm-up — available before any DMA lands
        wz = const_p.tile([128, 128], bf16, tag="wz")
        nc.vector.memset(wz[:], 0.0)

        # ---- prefetch, all on the 16-engine sync ring, in consumption order:
        # routed tokens + first weights lead (the PE's first real work), the
        # shared-expert inputs ride mid-stream where they have slack, and the
        # weight stream finishes ~4us earlier so the tail chain starts sooner.
        # Triggers cost ~0.65us each on the queue.
        xg = const_p.tile([128, E_LOC, HT, CAP], fp8e4, tag="xg")
        nc.sync.dma_start(xg[:, 0:2], xg_d.ap()[:, 0:2])

        wgs, wds = {}, {}

        def load_wgu(j):
            # two halves: gu(j) starts on h0-3 while h4-7 is still in flight
            wg = wgu_p.tile([128, HT, I2], fp8e4, tag="wgu", name=f"wg{j}")
            nc.sync.dma_start(wg[:, 0:HT // 2], wgu_d.ap()[j][:, 0:HT // 2])
            nc.sync.dma_start(wg[:, HT // 2:HT], wgu_d.ap()[j][:, HT // 2:HT])
            wgs[j] = wg

        def load_wd_pair(p):
            wd = wd_p.tile([128, 2, IT, H], fp8, tag="wd", name=f"wdp{p}")
            nc.sync.dma_start(wd[:], wdp_d.ap()[p])
            wds[2 * p] = wd[:, 0]
            wds[2 * p + 1] = wd[:, 1]

        load_wgu(0)
        load_wgu(1)
        nc.sync.dma_start(xg[:, 2:E_LOC], xg_d.ap()[:, 2:E_LOC])
        load_wd_pair(0)
        ident = const_p.tile([128, 128], bf16, tag="ident")
        nc.sync.dma_start(ident[:], id_d.ap()[:])
        load_wgu(2)
        xt = const_p.tile([128, HT, TSH], bf16, tag="xt")
        nc.sync.dma_start(xt[:], xt_d.ap()[:])
        swh_sb = const_p.tile([128, S, HT, 2 * ISHC], bf16, tag="swh")
        nc.sync.dma_start(swh_sb[:], swh_d.ap()[:])
        load_wgu(3)
        sdc_sb = const_p.tile([128, S, H], bf16, tag="sdc")
        nc.sync.dma_start(sdc_sb[:], sdc_d.ap()[:])
        load_wd_pair(1)
        load_wgu(4)
        load_wgu(5)
        load_wd_pair(2)
        load_wgu(6)
        load_wgu(7)
        load_wd_pair(3)

        def warm(n):
            # no-dependency matmuls on the memset tile: accumulate PE busy
            # time (p-state ramp) before the first DMAs even land
            ps = ps_y.tile([128, 512], f32, tag="y", name="ps_warm")
            for i in range(n):
                nc.tensor.matmul(ps[:, 0:128], wz[:], wz[:],
                                 start=(i == 0), stop=(i == n - 1))

        def emit_gu(j):
            # double-pumped gate_up: psum = (SCL_X x)·(SCL_W w), 2 k-tiles
            # per matmul, gate/up 512 cols each
            ps_g = ps_gu.tile([128, I], f32, tag="gu", name=f"psg{j}")
            ps_u = ps_gu.tile([128, I], f32, tag="gu", name=f"psu{j}")
            wg = wgs[j]
            for q in range(HT // 2):
                st = xg[:, j, 2 * q:2 * q + 2, :]
                nc.tensor.matmul(ps_g[:], st, wg[:, 2 * q:2 * q + 2, 0:I],
                                 start=(q == 0), stop=(q == HT // 2 - 1),
                                 perf_mode=PM.DoubleRow)
                nc.tensor.matmul(ps_u[:], st, wg[:, 2 * q:2 * q + 2, I:I2],
                                 start=(q == 0), stop=(q == HT // 2 - 1),
                                 perf_mode=PM.DoubleRow)
            sl = act_p.tile([128, I], f32, tag="sl", name=f"sl{j}")
            nc.scalar.activation(sl[:], ps_g[:], AF.Silu, scale=GU_DESCALE)
            hh = act_p.tile([128, I], bf16, tag="hh", name=f"hh{j}")
            nc.vector.tensor_mul(hh[:], sl[:], ps_u[:])  # = 16384 * hidden
            return hh

        def emit_dn(j, hh):
            # transpose hidden [tok, i] -> [i, tok] via PE; down-proj is
            # normal-rate e3m4 weights x bf16 hidden
            pt = ps_tr.tile([128, IT, CAP], bf16, tag="tr", name=f"pt{j}")
            for q in range(IT):
                nc.tensor.transpose(pt[:, q, :], hh[:, q * 128:(q + 1) * 128],
                                    ident[:])
            hhT = act_p.tile([128, IT, CAP], bf16, tag="hhT", name=f"hhT{j}")
            nc.vector.tensor_copy(hhT[:], pt[:])
            yo = y_p.tile([128, H], fp8, tag="y", name=f"yo{j}")
            wd = wds[j]
            for half in range(2):
                psy = ps_y.tile([128, 512], f32, tag="y", name=f"psy{j}_{half}")
                for it in range(IT):
                    nc.tensor.matmul(psy[:], hhT[:, it, :],
                                     wd[:, it, half * 512:(half + 1) * 512],
                                     start=(it == 0), stop=(it == IT - 1))
                if half == 0:
                    nc.scalar.activation(yo[:, 0:512], psy[:], AF.Copy,
                                         scale=Y_DESCALE)
                    if j == E_LOC - 1:
                        # last expert: store the first half early so only the
                        # second half trails the final matmuls
                        nc.sync.dma_start(y_d.ap()[j][:, 0:512], yo[:, 0:512])
                else:
                    nc.vector.tensor_scalar_mul(yo[:, 512:H], psy[:], Y_DESCALE)
            if j == E_LOC - 1:
                nc.sync.dma_start(y_d.ap()[j][:, 512:H], yo[:, 512:H])
            else:
                # gpsimd queue: keeps store triggers off the busy ACT queue
                nc.gpsimd.dma_start(y_d.ap()[j], yo[:])

        hc_t = {}

        def emit_shared_gu(s):
            # hc_t[s] = silu(gate)*up for shared expert s: [128 i-cols, TSH]
            pss = []
            for half in range(2):  # gate, up
                ps = ps_gu.tile([128, TSH], f32, tag="gu", name=f"shps{s}_{half}")
                for h in range(HT):
                    nc.tensor.matmul(
                        ps[:], swh_sb[:, s, h, half * ISHC:(half + 1) * ISHC],
                        xt[:, h, :], start=(h == 0), stop=(h == HT - 1))
                pss.append(ps)
            sl = act_p.tile([128, TSH], f32, tag="sl", name=f"slsh{s}")
            nc.scalar.activation(sl[:], pss[0][:], AF.Silu)
            hc = act_p.tile([128, TSH], bf16, tag=f"hc{s}", name=f"hc_t{s}")
            nc.vector.tensor_mul(hc[:], sl[:], pss[1][:])
            hc_t[s] = hc

        def emit_shared_down(tps):
            for tp in tps:
                so = sh_p.tile([128, H], bf16, tag="sh", name=f"so{tp}")
                for hh2 in range(2):
                    # ride the gu psum pool: keeps the 2-bank y rotation free
                    # for the routed down-proj pipeline
                    ps2 = ps_gu.tile([128, 512], f32, tag="gu",
                                     name=f"ps2_{tp}_{hh2}")
                    for s in range(S):
                        nc.tensor.matmul(
                            ps2[:], hc_t[s][:, tp * 128:(tp + 1) * 128],
                            sdc_sb[:, s, hh2 * 512:(hh2 + 1) * 512],
                            start=(s == 0), stop=(s == S - 1))
                    if hh2 == 0:
                        nc.vector.tensor_copy(so[:, 0:512], ps2[:])
                    else:
                        nc.scalar.activation(so[:, 512:H], ps2[:], AF.Copy)
                nc.gpsimd.dma_start(sh_d.ap()[tp * 128:(tp + 1) * 128, :], so[:])

        # ---- software pipeline: gu(j) || down(j-1); shared-expert compute
        # fronts the pipeline while the weight stream ramps. In the DMA-paced
        # tail, each dn is emitted before the next (possibly stalling) gu so
        # ready work never queues behind a weight wait.
        warm(64)
        hhs = {}
        hhs[0] = emit_gu(0)
        hhs[1] = emit_gu(1)
        emit_dn(0, hhs[0])
        hhs[2] = emit_gu(2)
        emit_dn(1, hhs[1])
        emit_shared_gu(0)
        hhs[3] = emit_gu(3)
        emit_shared_gu(1)
        emit_dn(2, hhs[2])
        hhs[4] = emit_gu(4)
        emit_shared_down([0, 1])
        emit_dn(3, hhs[3])
        hhs[5] = emit_gu(5)
        emit_shared_down([2, 3])
        emit_dn(4, hhs[4])
        emit_dn(5, hhs[5])
        hhs[6] = emit_gu(6)
        hhs[7] = emit_gu(7)
        emit_dn(6, hhs[6])
        emit_dn(7, hhs[7])
    nc.compile()
    return nc


def _route(x, gate_w):
    """Exact replica of the reference's grouped top-k gating, on CPU jax."""
    import jax
    import jax.numpy as jnp
    cpu = jax.devices("cpu")[0]
    with jax.default_device(cpu):
        xj = jax.device_put(np.asarray(x), cpu)
        gj = jax.device_put(np.asarray(gate_w), cpu)
        logits = xj @ gj.T
        t = logits.shape[0]
        group_size = E // G
        group_logits = logits.reshape(t, G, group_size)
        gw, gi = jax.lax.top_k(group_logits, TOPK_GROUP)
        gw = gw.reshape(t, G * TOPK_GROUP)
        gi = gi.reshape(t, G * TOPK_GROUP)
        topk_w, ti = jax.lax.top_k(gw, K)
        sel_group = ti // TOPK_GROUP
        expert_in_group = jnp.take_along_axis(gi, ti, axis=1)
        topk_idx = sel_group * group_size + expert_in_group
        topk_w = topk_w / (topk_w.sum(axis=-1, keepdims=True) + 1e-20)
    return np.asarray(topk_idx), np.asarray(topk_w).astype(np.float32)


def _expert_np(xrows, w_gu_e, w_d_e):
    """Reference expert math in numpy fp32 (overflow fallback only)."""
    g = xrows @ w_gu_e
    a = g[:, :I]
    hidden = (a / (1.0 + np.exp(-a))) * g[:, I:]
    return hidden @ w_d_e


def kernel(x, gate_w, w_gu, w_d, s_gu, s_d):
    global _CACHED_NC, LAST_RESULTS
    x = np.ascontiguousarray(np.asarray(x, dtype=np.float32))
    gate_w = np.ascontiguousarray(np.asarray(gate_w, dtype=np.float32))
    w_gu = np.asarray(w_gu, dtype=np.float32)
    w_d = np.asarray(w_d, dtype=np.float32)
    s_gu = np.asarray(s_gu, dtype=np.float32)
    s_d = np.asarray(s_d, dtype=np.float32)

    topk_idx, topk_w = _route(x, gate_w)

    flat_e = topk_idx.ravel()
    flat_t = np.repeat(np.arange(T), K)
    flat_w = topk_w.ravel()
    order = np.argsort(flat_e, kind="stable")
    sorted_t = flat_t[order]
    sorted_w = flat_w[order]
    counts = np.bincount(flat_e, minlength=E)
    starts = np.zeros(E + 1, np.int64)
    np.cumsum(counts, out=starts[1:])

    # token buffer [H, T] quantized to e4m3 (scaled by SCL_X)
    xTq = np.clip(x.T * SCL_X, -E4_MAX, E4_MAX).astype(E4)
    xg_all = np.zeros((N_CORES, 128, E_LOC, HT, CAP), E4)
    overflow = []
    for e in range(E):
        c, j = e // E_LOC, e % E_LOC
        n = int(counts[e])
        toks = sorted_t[starts[e]:starts[e] + n]
        nn = min(n, CAP)
        span = xg_all[c, :, j]  # [128, HT, CAP]
        span[:, :, :nn] = xTq[:, toks[:nn]].reshape(HT, 128, nn).transpose(1, 0, 2)
        if n > CAP:
            overflow.append((e, toks[CAP:],
                             sorted_w[starts[e] + CAP:starts[e] + n]))

    wgu_q = np.clip(w_gu * SCL_W, -E4_MAX, E4_MAX).astype(E4)
    wd_q = np.clip(w_d * SCL_WD, -E3_MAX, E3_MAX).astype(E3)
    wgu_s = wgu_q.reshape(E, HT, 128, I2).transpose(0, 2, 1, 3)  # [E,128,HT,2I]
    wd_s = wd_q.reshape(E, IT, 128, H).transpose(0, 2, 1, 3)     # [E,128,IT,H]

    if _CACHED_NC is None:
        _CACHED_NC = _build_nc()
    nc = _CACHED_NC

    s_gu_b = s_gu.astype(BF)
    s_d_b = s_d.astype(BF)
    xTb = np.ascontiguousarray(x.T.astype(BF))
    in_maps = []
    for c in range(N_CORES):
        tg, ig = c // 4, c % 4
        tsl = slice(tg * TSH, (tg + 1) * TSH)
        isl = slice(ig * ISHC, (ig + 1) * ISHC)
        # xt: this core's token half, partition-major
        xt_s = np.ascontiguousarray(
            xTb[:, tsl].reshape(HT, 128, TSH).transpose(1, 0, 2))
        # swh[s]: gate cols isl ++ up cols I+isl -> [128, S, HT, 256]
        swh = np.concatenate([s_gu_b[:, :, isl], s_gu_b[:, :, I:][:, :, isl]],
                             axis=2)                              # [S,H,2*ISHC]
        swh_s = np.ascontiguousarray(
            swh.reshape(S, HT, 128, 2 * ISHC).transpose(2, 0, 1, 3))
        # sdc: [128 i-rows, S, H]
        sdc = np.ascontiguousarray(
            s_d_b[:, isl, :].transpose(1, 0, 2))                  # [128,S,H]
        # down weights packed per pair: [4, 128, 2, IT, H]
        wd_core = wd_s[c * E_LOC:(c + 1) * E_LOC]
        wdp = np.ascontiguousarray(
            wd_core.reshape(E_LOC // 2, 2, 128, IT, H).transpose(0, 2, 1, 3, 4))
        in_maps.append({
            "xg": xg_all[c],
            "wgu": np.ascontiguousarray(wgu_s[c * E_LOC:(c + 1) * E_LOC]),
            "wdp": wdp,
            "xt": xt_s,
            "swh": swh_s,
            "sdc": sdc,
            "ident": np.eye(128, dtype=BF),
        })

    res = run_bass_kernel_spmd(nc, in_maps, list(range(N_CORES)), trace=_TRACE)
    LAST_RESULTS = res

    out = np.zeros((T, H), np.float32)
    for c in range(N_CORES):
        tg = c // 4
        out[tg * TSH:(tg + 1) * TSH] += res.results[c]["sh"].astype(np.float32)

    # routed: weighted rows placed at their flat (t, k) position, sum over k
    routed_flat = np.zeros((T * K, H), np.float32)
    for e in range(E):
        c, j = e // E_LOC, e % E_LOC
        n = min(int(counts[e]), CAP)
        ye = res.results[c]["y"][j].astype(np.float32) * (1.0 / SCL_Y)
        ws = sorted_w[starts[e]:starts[e] + n]
        routed_flat[order[starts[e]:starts[e] + n]] = ye[:n] * ws[:, None]
    for e, toks, ws in overflow:
        y_extra = _expert_np(x[toks], w_gu[e], w_d[e]) * ws[:, None]
        routed_flat[order[starts[e] + CAP:starts[e] + CAP + len(toks)]] = y_extra
    out += routed_flat.reshape(T, K, H).sum(axis=1)
    return out
